# revision 61
# baseline (speedup 1.0000x reference)
"""Trainium2 Bass kernel v4 for nn_Decoder.

v3 -> v4: host sorts each image's points by output row (y0) and packs
them into chunks whose taps span <= 8 consecutive rows.  The image is
accumulated TRANSPOSED (cols as psum partitions), so the per-chunk
matmuls stream only an 8-wide moving operand:

  out_T[h][:, r0_c : r0_c+8] += x_k[:, h*128:(h+1)*128]^T @ t[:, c*8:(c+1)*8]

where t (the per-point y-tap weights * value, width 8) is precomputed
on the host and DMAed in, and r0_c comes from an int32 plane via a PE
register (dynamic PSUM slice).  Per chunk the device only builds the
x-side tensors: 3 of 4 chunks as two DVE one-hots (is_equal * weight),
1 of 4 as an ACT hat (Abs; Relu(1-.)).  PSUM is pre-zeroed per image
and all matmuls accumulate with start=False + skip_group_check.

The DFT/CTF chain is unchanged (all DFT operators are symmetric); the
CTF ships transposed and the final output is un-transposed on host.
"""

import os

import bass_rust
import ml_dtypes
import numpy as np

import concourse.bass as bass
import concourse.mybir as mybir
import concourse.tile as tile_mod
from concourse.bass_utils import run_bass_kernel_spmd
from concourse.tile import TileContext
from concourse.vector_clock import ScopedClock

B = 32
N = 100000
XS = 256
KSIZE = 5
N_CORES = 8
IMG_PER_CORE = B // N_CORES
W = 8  # row window width per chunk
G = int(os.environ.get("BASS4_G", "2"))  # chunks per r0 group
# static chunk slots: G * worst-case groups (ceil(N/(128G)) full + 37 splits)
CH = {1: 820, 2: 860, 4: 940, 8: 1200}[G]
F32 = mybir.dt.float32
F32R = mybir.dt.float32r
BF16 = mybir.dt.bfloat16
I32 = mybir.dt.int32
AF = mybir.ActivationFunctionType
ALU = mybir.AluOpType
NPBF16 = ml_dtypes.bfloat16

STAGE_DT = F32R
# every OH_DEN-th chunk builds its x weights as a DVE one-hot pair
# (relieving ACT); measured slower than pure hybrid on HW, so disabled
OH_DEN = int(os.environ.get("BASS4_OH_DEN", "1000000"))
BUFS = int(os.environ.get("BASS4_BUFS", "16"))
STRIP = os.environ.get("BASS4_STRIP", "1") == "1"
BIG = 12582912.0  # 1.5 * 2**23

# ---------------------------------------------------------------------------
_PATCHED = False


def _patch_tile_drain():
    global _PATCHED
    if _PATCHED:
        return
    _PATCHED = True

    def _drain_and_barrier(self, tick_clock, wait_clock):
        probe = self.nc.sync.nop(nofuse=True, hint="drain_wait_probe")
        wait_clock.add_sem_waits(
            probe.ins, ScopedClock({None: tick_clock.global_clock})
        )
        si = probe.ins.sync_info
        waits = list(si.on_wait) if si is not None else []
        probe.ins.sync_info = mybir.SyncInfo(on_wait=waits[:1], on_update=[])
        for w in waits[1:]:
            n = self.nc.sync.nop(nofuse=True, hint="drain_wait_extra")
            n.ins.sync_info = mybir.SyncInfo(on_wait=[w], on_update=[])
        self.nc.sync.drain()
        self.nc.all_engine_barrier()
        assert self.sems is not None
        popped = self.nc._tile_sem_poison_stack.pop()
        assert popped is self._sem_poison
        self.nc.clear_and_free_semaphores(list(self.sems.allocated().values()))
        self.nc.all_engine_barrier()

    tile_mod.TileContext._drain_and_barrier = _drain_and_barrier


_COMMIT_PATCHED = False


def _patch_commit_free_tmps():
    """Free the per-matmul dynamic-AP lowering temps so TileContext exit
    doesn't exhaust the 54-register PE file on ~3k unrolled dynamic
    matmuls.  The freed names are recycled by later lowerings; per-engine
    in-order execution keeps the def-use chains correct, and the strip
    pass below removes all of these ALUs anyway."""
    global _COMMIT_PATCHED
    if _COMMIT_PATCHED:
        return
    _COMMIT_PATCHED = True
    orig = tile_mod.TileContext._commit_instruction

    def patched(self, inst, lazy_reg_writes=True):
        orig(self, inst, lazy_reg_writes)
        if isinstance(inst, mybir.InstMatmult) and inst.outs:
            o = inst.outs[0]
            if isinstance(o, mybir.RegisterAccessPattern) and "_tmp_" in o.regref:
                base, n = o.regref.rsplit("_", 1)
                n = int(n)
                for k in (n, n - 2):
                    try:
                        self.nc.free_register(
                            bass_rust.RegisterHandle(
                                f"{base}_{k}", mybir.EngineType.PE
                            )
                        )
                    except Exception:
                        pass

    tile_mod.TileContext._commit_instruction = patched


def _strip_dyn_alus(nc, pin_names):
    """Rewrite dynamic matmuls to reference the pinned registers directly.

    Lowering turned each `ds(val, W)` into `tmpA = P*4; tmpB = tmpA + C;
    mm(RAP regref=tmpB)`.  The host ships P's value already as the
    absolute byte address (r0*4 + CBASE), so both ALUs are deleted, the
    RAP is re-targeted at P, and any per-mm base difference (C - CBASE,
    e.g. the second PSUM half) is folded into the RAP's static element
    offset.  Returns {pin_name: CBASE}."""
    # pass 1: find each dyn matmul's (alu1, alu2, pin, C)
    matches = []
    for fn in nc.m.functions:
        for bb in fn.blocks:
            writer = {}
            for ins in bb.instructions:
                if isinstance(ins, mybir.InstRegisterAlu):
                    writer[ins.outs[0].regref] = ins
                    continue
                if not (isinstance(ins, mybir.InstMatmult) and ins.outs):
                    continue
                o = ins.outs[0]
                if not (isinstance(o, mybir.RegisterAccessPattern)
                        and "_tmp_" in o.regref):
                    continue
                alu2 = writer[o.regref]
                in_reg = [a for a in alu2.ins
                          if isinstance(a, mybir.RegisterAccess)]
                in_imm = [a for a in alu2.ins
                          if isinstance(a, mybir.ImmediateValue)]
                alu1 = writer[in_reg[0].regref]
                p_reg = [a for a in alu1.ins
                         if isinstance(a, mybir.RegisterAccess)][0].regref
                assert p_reg in pin_names, f"unexpected base reg {p_reg}"
                matches.append((bb, o, alu1, alu2, p_reg,
                                int(in_imm[0].value)))
    consts = {}
    for _, _, _, _, p_reg, c_val in matches:
        consts[p_reg] = min(consts.get(p_reg, c_val), c_val)
    dead_by_bb = {}
    for bb, o, alu1, alu2, p_reg, c_val in matches:
        delta = c_val - consts[p_reg]
        assert delta == 0, f"psum base mismatch for {p_reg}: {delta}"
        o.regref = p_reg
        dead_by_bb.setdefault(id(bb), (bb, set()))[1].update(
            (alu1.name, alu2.name))
    for bb, dead in dead_by_bb.values():
        bb.instructions = [i for i in bb.instructions if i.name not in dead]
    return consts


FOLD_LDW = os.environ.get("BASS4_FOLD_LDW", "0") == "1"


def _fold_ldweights(nc):
    """Delete standalone InstLdweights for the tiny dynamic matmuls and mark
    the paired matmul self-loading, halving PE queue slots.  Waits on the
    deleted Ldweights are merged into the matmul's sync_info."""
    n = 0
    for fn in nc.m.functions:
        for bb in fn.blocks:
            pending = {}  # weights AP key -> ldweights inst
            dead = set()
            for ins in bb.instructions:
                if isinstance(ins, mybir.InstLdweights):
                    key = str(ins.ins[0])
                    pending[key] = ins
                    continue
                if isinstance(ins, mybir.InstMatmult) and len(ins.ins) == 2:
                    key = str(ins.ins[1])
                    ldw = pending.pop(key, None)
                    if ldw is None:
                        continue
                    lsi = ldw.sync_info
                    msi = ins.sync_info
                    waits = list(lsi.on_wait) if lsi else []
                    if msi is not None:
                        waits += list(msi.on_wait)
                    ups = list(msi.on_update) if msi else []
                    if lsi is not None:
                        ups += list(lsi.on_update)
                    ins.sync_info = mybir.SyncInfo(on_wait=waits,
                                                  on_update=ups)
                    ins.ldweights = True
                    dead.add(ldw.name)
                    n += 1
            if dead:
                bb.instructions = [
                    i for i in bb.instructions if i.name not in dead
                ]
    return n


def _split_excess_waits(nc):
    n = 0
    for fn in nc.m.functions:
        for bb in fn.blocks:
            il = bb.instructions
            out = []
            changed = False
            for ins in il:
                si = ins.sync_info
                if si is not None and len(si.on_wait) > 1:
                    waits = list(si.on_wait)
                    for w in waits[:-1]:
                        n += 1
                        nop = mybir.InstNoOp(
                            name=f"I-waitsplit-{n}", ins=[], outs=[]
                        )
                        nop.engine = ins.engine
                        nop.sync_info = mybir.SyncInfo(
                            on_wait=[w], on_update=[]
                        )
                        nc.register_instruction(nop)
                        out.append(nop)
                    ins.sync_info = mybir.SyncInfo(
                        on_wait=[waits[-1]], on_update=list(si.on_update)
                    )
                    changed = True
                out.append(ins)
            if changed:
                bb.instructions = out


# ---------------------------------------------------------------------------
# Host-side math


def _rot6d(alignment):
    a1, a2 = alignment[:, :3], alignment[:, 3:]
    b1 = a1 / (np.linalg.norm(a1, axis=-1, keepdims=True) + 1e-8)
    a2p = a2 - np.sum(b1 * a2, axis=-1, keepdims=True) * b1
    b2 = a2p / (np.linalg.norm(a2p, axis=-1, keepdims=True) + 1e-8)
    b3 = np.cross(b1, b2)
    return np.stack([b1, b2, b3], axis=1)


def _conv_matrix(g1, n):
    m = np.zeros((n, n), np.float64)
    for i in range(n):
        for u in range(KSIZE):
            j = i + u - KSIZE // 2
            if 0 <= j < n:
                m[i, j] += g1[u]
    return m


DFT_NAMES = [
    "wgy_t_r", "wgy_t_i",
    "wgx_t_r", "wgx_t_i", "wgx_t_in",
    "wit_r", "wit_i", "wit_in",
]


def _dft_consts(gauss_kernel):
    u, s, vt = np.linalg.svd(gauss_kernel.astype(np.float64))
    gy = np.sqrt(s[0]) * u[:, 0]
    gx = np.sqrt(s[0]) * vt[0, :]
    if gy[KSIZE // 2] < 0:
        gy, gx = -gy, -gx
    k = np.arange(XS)
    w = np.exp(-2j * np.pi * np.outer(k, k) / XS)
    winv = np.conj(w) / XS
    # NOTE: the kernel computes the TRANSPOSED image pipeline; since the
    # conv/DFT operators are symmetric matrices this only swaps the roles
    # of gx and gy below.
    wgy_t = (w @ _conv_matrix(gx, XS)).T
    wgx_t = (w @ _conv_matrix(gy, XS)).T
    wit = winv.T
    consts = {
        "wgy_t_r": np.real(wgy_t),
        "wgy_t_i": np.imag(wgy_t),
        "wgx_t_r": np.real(wgx_t),
        "wgx_t_i": np.imag(wgx_t),
        "wgx_t_in": -np.imag(wgx_t),
        "wit_r": np.real(wit),
        "wit_i": np.imag(wit),
        "wit_in": -np.imag(wit),
    }
    return {
        name: np.ascontiguousarray(m.reshape(2, 128, XS).astype(np.float32))
        for name, m in consts.items()
    }


# ---------------------------------------------------------------------------
# Device program

_PROGRAM = None


_R0_CONSTS = None  # {h: psum byte base}, set by build_program


def build_program(img_per_core=IMG_PER_CORE, n_chunks=CH):
    global _R0_CONSTS
    _patch_tile_drain()
    _patch_commit_free_tmps()
    nc = bass.Bass()

    pxp = nc.declare_dram_parameter("pxp", [img_per_core, 128, n_chunks], F32,
                                    isOutput=False)
    tpl = nc.declare_dram_parameter("tpl", [img_per_core, 128, n_chunks * W],
                                    BF16, isOutput=False)
    r0p = nc.declare_dram_parameter(
        "r0p", [img_per_core, 1, 2 * (n_chunks // G)], I32, isOutput=False
    )
    iota16 = nc.declare_dram_parameter("iota16", [2, 128, XS], BF16,
                                       isOutput=False)
    iota32 = nc.declare_dram_parameter("iota32", [128, XS], F32,
                                       isOutput=False)
    iota2 = nc.declare_dram_parameter("iota2", [128, 2 * XS], BF16,
                                      isOutput=False)
    ctf = nc.declare_dram_parameter(
        "ctf", [img_per_core, 2, 128, XS], BF16, isOutput=False
    )
    dft = {
        name: nc.declare_dram_parameter(name, [2, 128, XS], STAGE_DT,
                                        isOutput=False)
        for name in DFT_NAMES
    }
    out = nc.declare_dram_parameter(
        "out", [img_per_core, XS, XS], BF16, isOutput=True
    )

    pins = [nc.tensor.alloc_register(f"dynP{h}") for h in range(2)]
    pin_names = {p.name for p in pins}
    pvals = [
        bass.make_scalar_value(
            bass.RegisterHandles(p), min_val=0, max_val=XS - W
        )
        for p in pins
    ]

    with TileContext(nc) as tc:
        with (
            tc.tile_pool(name="const", bufs=1) as cpool,
            tc.tile_pool(name="planes", bufs=2) as ppool,
            tc.tile_pool(name="r0pool", bufs=max(img_per_core, 2)) as rpool,
            tc.tile_pool(name="deriv", bufs=2) as dpool,
            tc.tile_pool(name="build", bufs=BUFS) as bpool,
            tc.tile_pool(name="stage", bufs=2) as spool,
            tc.tile_pool(name="psumS", bufs=1, space="PSUM") as qspool,
            tc.tile_pool(name="psumA", bufs=3, space="PSUM") as qapool,
            tc.tile_pool(name="psumB", bufs=3, space="PSUM") as qbpool,
        ):
            io16 = cpool.tile([128, XS], BF16, tag="io16", name="io16")
            nc.sync.dma_start(out=io16[:], in_=iota16[0])
            io16m1 = cpool.tile([128, XS], BF16, tag="io16m1", name="io16m1")
            nc.sync.dma_start(out=io16m1[:], in_=iota16[1])
            io32 = cpool.tile([128, XS], F32, tag="io32", name="io32")
            nc.sync.dma_start(out=io32[:], in_=iota32[:])
            io2 = cpool.tile([128, 2 * XS], BF16, tag="io2", name="io2")
            nc.sync.dma_start(out=io2[:], in_=iota2[:])
            dft_t = {}
            for name in DFT_NAMES:
                for kc in range(2):
                    t = cpool.tile([128, XS], STAGE_DT, tag=f"{name}{kc}",
                                   name=f"c_{name}{kc}")
                    nc.sync.dma_start(out=t[:], in_=dft[name][kc])
                    dft_t[name, kc] = t

            for b in range(img_per_core):
                px_t = ppool.tile([128, n_chunks], F32, tag="px", name="px_t")
                nc.sync.dma_start(out=px_t[:], in_=pxp[b])
                t_t = ppool.tile([128, n_chunks * W], BF16, tag="tpl",
                                 name="t_t")
                nc.sync.dma_start(out=t_t[:], in_=tpl[b])
                r0_t = rpool.tile([1, 2 * (n_chunks // G)], I32, tag="r0",
                                  name="r0_t")
                nc.sync.dma_start(out=r0_t[:], in_=r0p[b])
                ctf_t = [ppool.tile([128, XS], BF16, tag=f"ctf{h}",
                                    name=f"ctf_t{h}") for h in range(2)]
                for h in range(2):
                    nc.sync.dma_start(out=ctf_t[h][:], in_=ctf[b, h])

                # ---- splat into transposed image (cols as partitions) ----
                img_ps = [
                    qspool.tile([128, XS], F32, tag=f"psS{h}",
                                name=f"img_ps{h}")
                    for h in range(2)
                ]
                for h in range(2):
                    nc.vector.memset(img_ps[h][:], 0.0)

                # ---- derive pxn = -px (ACT Abs bias) and, for the one-hot
                # chunks, x0 = floor(px) plus NEGATED fractional weights
                # (the t plane ships negated)
                pxn = dpool.tile([128, n_chunks], F32, tag="pxn", name="pxn")
                nc.vector.tensor_scalar(pxn[:], px_t[:], -1.0, None, ALU.mult)
                if OH_DEN <= n_chunks:
                    q = dpool.tile([128, n_chunks], F32, tag="q", name="q")
                    nc.vector.tensor_scalar(q[:], px_t[:], 0.499, BIG,
                                            ALU.subtract, ALU.add)
                    x0 = dpool.tile([128, n_chunks], F32, tag="x0",
                                    name="x0")
                    nc.vector.tensor_scalar(x0[:], q[:], BIG, None,
                                            ALU.subtract)
                    b1n = dpool.tile([128, n_chunks], F32, tag="b1n",
                                     name="b1n")
                    nc.vector.tensor_sub(b1n[:], x0[:], px_t[:])
                    b0n = dpool.tile([128, n_chunks], F32, tag="b0n",
                                     name="b0n")
                    nc.vector.tensor_scalar(b0n[:], b1n[:], -1.0, -1.0,
                                            ALU.mult, ALU.add)

                for c in range(n_chunks):
                    if c % OH_DEN == OH_DEN - 1:
                        # DVE one-hot pair with negated weights: the
                        # double negative vs the negated t plane cancels.
                        x1 = bpool.tile([128, XS], BF16, tag="x1", name="x1")
                        nc.vector.tensor_scalar(
                            x1[:], io16[:], x0[:, c : c + 1],
                            b0n[:, c : c + 1], ALU.is_equal, ALU.mult,
                        )
                        x2 = bpool.tile([128, XS], BF16, tag="x2", name="x2")
                        nc.vector.tensor_scalar(
                            x2[:], io16m1[:], x0[:, c : c + 1],
                            b1n[:, c : c + 1], ALU.is_equal, ALU.mult,
                        )
                        xts = [x1, x2]
                    else:
                        # hybrid hat: ACT computes a=|io-px|; one DVE op
                        # computes min(a,1)-1 = -hat; negated t plane
                        # restores the sign in the matmul.
                        xa = bpool.tile([128, XS], BF16, tag="xa", name="xa")
                        nc.scalar.activation(
                            xa[:], io32[:], AF.Abs,
                            bias=pxn[:, c : c + 1], scale=1.0,
                        )
                        xh = bpool.tile([128, XS], BF16, tag="xh", name="xh")
                        nc.vector.tensor_scalar(
                            xh[:], xa[:], 1.0, 1.0, ALU.min, ALU.subtract,
                        )
                        xts = [xh]
                    if c % G == 0:
                        g = c // G
                        nc.tensor.reg_load(
                            pins, r0_t[0:1, 2 * g : 2 * g + 2])
                    tcol = t_t[:, c * W : (c + 1) * W]
                    for xt in xts:
                        for h in range(2):
                            nc.tensor.matmul(
                                img_ps[h][:, bass.ds(pvals[h], W)],
                                xt[:, h * 128 : (h + 1) * 128],
                                tcol,
                                start=False,
                                stop=False,
                                skip_group_check=True,
                            )

                img_sb = [
                    spool.tile([128, XS], STAGE_DT, tag=f"isb{h}",
                               name=f"isb{h}") for h in range(2)
                ]
                for h in range(2):
                    nc.vector.tensor_copy(img_sb[h][:], img_ps[h][:])

                # ---- DFT chain (operates on transposed image) ----
                def product(terms, tag, ps_tag, mult_by=None):
                    res = []
                    for ho in range(2):
                        qp = qapool if ps_tag == "psA" else qbpool
                        ps = qp.tile([128, XS], F32, tag=ps_tag,
                                     name=f"ps_{tag}{ho}")
                        nmm = 2 * len(terms)
                        i = 0
                        for lhs_tiles, rhs_name in terms:
                            for kc in range(2):
                                nc.tensor.matmul(
                                    ps[:],
                                    lhs_tiles[kc][
                                        :, ho * 128 : (ho + 1) * 128
                                    ],
                                    dft_t[rhs_name, kc][:],
                                    start=(i == 0),
                                    stop=(i == nmm - 1),
                                )
                                i += 1
                        sb = spool.tile([128, XS], STAGE_DT,
                                        tag=f"sb{tag}{ho}",
                                        name=f"sb{tag}{ho}")
                        if mult_by is not None:
                            nc.vector.tensor_mul(sb[:], ps[:],
                                                 mult_by[ho][:])
                        else:
                            nc.vector.tensor_copy(sb[:], ps[:])
                        res.append(sb)
                    return res

                ar = product([(img_sb, "wgy_t_r")], "ar", "psB")
                ai = product([(img_sb, "wgy_t_i")], "ai", "psB")
                fr = product(
                    [(ar, "wgx_t_r"), (ai, "wgx_t_in")], "fr", "psA",
                    mult_by=ctf_t,
                )
                fi = product(
                    [(ar, "wgx_t_i"), (ai, "wgx_t_r")], "fi", "psA",
                    mult_by=ctf_t,
                )
                br = product([(fr, "wit_r"), (fi, "wit_in")], "br", "psB")
                bi = product([(fr, "wit_i"), (fi, "wit_r")], "bi", "psB")
                for ho in range(2):
                    ps = qapool.tile([128, XS], F32, tag="psA",
                                    name=f"ps_o{ho}")
                    i = 0
                    for lhs_tiles, rhs_name in [(br, "wit_r"), (bi, "wit_in")]:
                        for kc in range(2):
                            nc.tensor.matmul(
                                ps[:],
                                lhs_tiles[kc][:, ho * 128 : (ho + 1) * 128],
                                dft_t[rhs_name, kc][:],
                                start=(i == 0),
                                stop=(i == 3),
                            )
                            i += 1
                    osb = spool.tile([128, XS], BF16, tag=f"osb{ho}",
                                     name=f"osb{ho}")
                    nc.vector.tensor_copy(osb[:], ps[:])
                    nc.sync.dma_start(
                        out=out[b, ho * 128 : (ho + 1) * 128, :], in_=osb[:]
                    )
    if STRIP:
        consts = _strip_dyn_alus(nc, pin_names)
        _R0_CONSTS = {h: consts[pins[h].name] for h in range(2)}
    else:
        _R0_CONSTS = {0: 0, 1: 0}  # registers hold element offsets
    if FOLD_LDW:
        _fold_ldweights(nc)
    _split_excess_waits(nc)
    return nc


# ---------------------------------------------------------------------------
# Host prep


def _prep_image(px, py, values, n_chunks):
    """Sort one image's points by y0, pack into row-window chunks.

    Returns px_pl [128, CH] f32, t_pl [128, CH*W] bf16, r0_pl [1, CH] i32.
    """
    y0 = np.floor(py).astype(np.int32)
    fy = (py - y0).astype(np.float32)
    order = np.argsort(y0, kind="stable")
    y0s = y0[order]
    pxs = px[order].astype(np.float32)
    fys = fy[order]
    vs = values[order].astype(np.float32)

    n = y0s.shape[0]
    # first index whose y0 >= y0s[i] + (W - 1): a group starting at i must
    # end before it so taps stay inside [y0s[i], y0s[i] + W)
    limit = np.searchsorted(y0s, y0s + (W - 1), side="left")
    # greedy G-chunk groups sharing one r0
    gstarts = []
    gends = []
    s = 0
    while s < n:
        e = min(s + 128 * G, int(limit[s]), n)
        gstarts.append(s)
        gends.append(e)
        s = e
    ngroups = len(gstarts)
    n_groups_max = n_chunks // G
    assert ngroups <= n_groups_max, (
        f"chunking overflow: {ngroups} > {n_groups_max}")

    # split each group into G chunk slots
    starts, ends, group_of_chunk = [], [], []
    for g, (gs, ge) in enumerate(zip(gstarts, gends)):
        for k in range(G):
            cs = min(gs + 128 * k, ge)
            ce = min(cs + 128, ge)
            starts.append(cs)
            ends.append(ce)
            group_of_chunk.append(g)
    nch = len(starts)

    starts_a = np.asarray(starts, np.int64)
    ends_a = np.asarray(ends, np.int64)
    lens = ends_a - starts_a
    chunk_id = np.repeat(np.arange(nch, dtype=np.int64), lens)
    slot = np.arange(n, dtype=np.int64) - np.repeat(starts_a, lens)

    gr0 = np.minimum(y0s[np.asarray(gstarts, np.int64)],
                     XS - W).astype(np.int32)
    # interleaved pairs (h0, h1) per GROUP: absolute PSUM byte addresses
    # (STRIP mode) or plain element offsets (no-strip mode)
    r0_pl = np.zeros((1, 2 * n_groups_max), np.int32)
    for h in range(2):
        r0_pl[0, h::2] = _R0_CONSTS[h]
        r0_pl[0, h : 2 * ngroups : 2] += gr0 * (4 if STRIP else 1)

    px_pl = np.zeros((128, n_chunks), np.float32)
    px_pl[slot, chunk_id] = pxs

    r0 = gr0[np.asarray(group_of_chunk, np.int64)]
    j = (y0s - r0[chunk_id]).astype(np.int64)
    w0 = (1.0 - fys) * vs
    w1 = fys * vs
    t3 = np.zeros((128, n_chunks, W), np.float32)
    t3[slot, chunk_id, j] = w0
    m = j + 1 <= W - 1
    t3[slot[m], chunk_id[m], j[m] + 1] = w1[m]
    # negated: the device computes -hat on the x side (see build_program)
    t_pl = np.ascontiguousarray(-t3.reshape(128, n_chunks * W)).astype(NPBF16)
    return px_pl, t_pl, r0_pl


def _prep_host(alignment, shifts, coords, values, gauss_kernel, ctf,
               img_per_core=IMG_PER_CORE, n_chunks=CH):
    rot = _rot6d(alignment.astype(np.float64))
    nb = rot.shape[0]
    half = XS // 2
    pc = coords.astype(np.float64)
    vals = values.astype(np.float32)

    pxs_all = []
    tpl_all = []
    r0_all = []
    for b in range(nb):
        px = pc @ rot[b, 0] + (float(shifts[b, 0]) + half)
        py = pc @ rot[b, 1] + (float(shifts[b, 1]) + half)
        np.clip(px, 0.0, XS - 1.0, out=px)
        np.clip(py, 0.0, XS - 1.0, out=py)
        px_pl, t_pl, r0_pl = _prep_image(
            px.astype(np.float32), py.astype(np.float32), vals, n_chunks
        )
        pxs_all.append(px_pl)
        tpl_all.append(t_pl)
        r0_all.append(r0_pl)

    iota = np.arange(XS, dtype=np.float64)
    iota16 = np.ascontiguousarray(
        np.stack([
            np.broadcast_to(iota, (128, XS)),
            np.broadcast_to(iota - 1.0, (128, XS)),
        ]).astype(NPBF16)
    )
    iota32 = np.ascontiguousarray(
        np.broadcast_to(iota, (128, XS)).astype(np.float32)
    )
    iota2 = np.ascontiguousarray(
        np.broadcast_to(np.concatenate([iota, iota]), (128, 2 * XS))
        .astype(NPBF16)
    )
    consts = _dft_consts(gauss_kernel)
    # transposed-image pipeline -> ship the CTF transposed
    cs = np.fft.ifftshift(ctf.astype(np.float32), axes=(-2, -1))
    cs = np.ascontiguousarray(np.transpose(cs, (0, 2, 1)))
    cs = np.ascontiguousarray(cs.reshape(nb, 2, 128, XS)).astype(NPBF16)

    n_cores = nb // img_per_core
    in_maps = []
    for core in range(n_cores):
        sl = slice(core * img_per_core, (core + 1) * img_per_core)
        m = {
            "pxp": np.ascontiguousarray(np.stack(pxs_all[sl])),
            "tpl": np.ascontiguousarray(np.stack(tpl_all[sl])),
            "r0p": np.ascontiguousarray(np.stack(r0_all[sl])),
            "iota16": iota16, "iota32": iota32, "iota2": iota2,
            "ctf": np.ascontiguousarray(cs[sl]),
        }
        m.update(consts)
        in_maps.append(m)
    return in_maps


def kernel(alignment, shifts, coords, values, gauss_kernel, ctf):
    global _PROGRAM
    if _PROGRAM is None:
        _PROGRAM = build_program()
    in_maps = _prep_host(
        np.asarray(alignment), np.asarray(shifts), np.asarray(coords),
        np.asarray(values), np.asarray(gauss_kernel), np.asarray(ctf),
    )
    res = run_bass_kernel_spmd(_PROGRAM, in_maps, list(range(N_CORES)))
    out_t = np.concatenate([r["out"] for r in res.results], axis=0)
    return np.ascontiguousarray(
        np.transpose(out_t, (0, 2, 1))
    ).astype(np.float32)


# revision 62
# speedup vs baseline: 1.1252x; 1.1252x over previous
"""Trainium2 Bass kernel v4 for nn_Decoder.

v3 -> v4: host sorts each image's points by output row (y0) and packs
them into chunks whose taps span <= 8 consecutive rows.  The image is
accumulated TRANSPOSED (cols as psum partitions), so the per-chunk
matmuls stream only an 8-wide moving operand:

  out_T[h][:, r0_c : r0_c+8] += x_k[:, h*128:(h+1)*128]^T @ t[:, c*8:(c+1)*8]

where t (the per-point y-tap weights * value, width 8) is precomputed
on the host and DMAed in, and r0_c comes from an int32 plane via a PE
register (dynamic PSUM slice).  Per chunk the device only builds the
x-side tensors: 3 of 4 chunks as two DVE one-hots (is_equal * weight),
1 of 4 as an ACT hat (Abs; Relu(1-.)).  PSUM is pre-zeroed per image
and all matmuls accumulate with start=False + skip_group_check.

The DFT/CTF chain is unchanged (all DFT operators are symmetric); the
CTF ships transposed and the final output is un-transposed on host.
"""

import os

import bass_rust
import ml_dtypes
import numpy as np

import concourse.bass as bass
import concourse.mybir as mybir
import concourse.tile as tile_mod
from concourse.bass_utils import run_bass_kernel_spmd
from concourse.tile import TileContext
from concourse.vector_clock import ScopedClock

B = 32
N = 100000
XS = 256
KSIZE = 5
N_CORES = 8
IMG_PER_CORE = B // N_CORES
W = 8  # row window width per chunk
G = int(os.environ.get("BASS4_G", "2"))  # chunks per r0 group
# static chunk slots: G * worst-case groups (ceil(N/(128G)) full + 37 splits)
CH = {1: 820, 2: 860, 4: 940, 8: 1200}[G]
F32 = mybir.dt.float32
F32R = mybir.dt.float32r
BF16 = mybir.dt.bfloat16
I32 = mybir.dt.int32
AF = mybir.ActivationFunctionType
ALU = mybir.AluOpType
NPBF16 = ml_dtypes.bfloat16

STAGE_DT = F32R
# every OH_DEN-th chunk builds its x weights as a DVE one-hot pair
# (relieving ACT); measured slower than pure hybrid on HW, so disabled
OH_DEN = int(os.environ.get("BASS4_OH_DEN", "1000000"))
BUFS = int(os.environ.get("BASS4_BUFS", "32"))
STRIP = os.environ.get("BASS4_STRIP", "1") == "1"
BIG = 12582912.0  # 1.5 * 2**23

# ---------------------------------------------------------------------------
_PATCHED = False


def _patch_tile_drain():
    global _PATCHED
    if _PATCHED:
        return
    _PATCHED = True

    def _drain_and_barrier(self, tick_clock, wait_clock):
        probe = self.nc.sync.nop(nofuse=True, hint="drain_wait_probe")
        wait_clock.add_sem_waits(
            probe.ins, ScopedClock({None: tick_clock.global_clock})
        )
        si = probe.ins.sync_info
        waits = list(si.on_wait) if si is not None else []
        probe.ins.sync_info = mybir.SyncInfo(on_wait=waits[:1], on_update=[])
        for w in waits[1:]:
            n = self.nc.sync.nop(nofuse=True, hint="drain_wait_extra")
            n.ins.sync_info = mybir.SyncInfo(on_wait=[w], on_update=[])
        self.nc.sync.drain()
        self.nc.all_engine_barrier()
        assert self.sems is not None
        popped = self.nc._tile_sem_poison_stack.pop()
        assert popped is self._sem_poison
        self.nc.clear_and_free_semaphores(list(self.sems.allocated().values()))
        self.nc.all_engine_barrier()

    tile_mod.TileContext._drain_and_barrier = _drain_and_barrier


_COMMIT_PATCHED = False


def _patch_commit_free_tmps():
    """Free the per-matmul dynamic-AP lowering temps so TileContext exit
    doesn't exhaust the 54-register PE file on ~3k unrolled dynamic
    matmuls.  The freed names are recycled by later lowerings; per-engine
    in-order execution keeps the def-use chains correct, and the strip
    pass below removes all of these ALUs anyway."""
    global _COMMIT_PATCHED
    if _COMMIT_PATCHED:
        return
    _COMMIT_PATCHED = True
    orig = tile_mod.TileContext._commit_instruction

    def patched(self, inst, lazy_reg_writes=True):
        orig(self, inst, lazy_reg_writes)
        if isinstance(inst, mybir.InstMatmult) and inst.outs:
            o = inst.outs[0]
            if isinstance(o, mybir.RegisterAccessPattern) and "_tmp_" in o.regref:
                base, n = o.regref.rsplit("_", 1)
                n = int(n)
                for k in (n, n - 2):
                    try:
                        self.nc.free_register(
                            bass_rust.RegisterHandle(
                                f"{base}_{k}", mybir.EngineType.PE
                            )
                        )
                    except Exception:
                        pass

    tile_mod.TileContext._commit_instruction = patched


def _strip_dyn_alus(nc, pin_names):
    """Rewrite dynamic matmuls to reference the pinned registers directly.

    Lowering turned each `ds(val, W)` into `tmpA = P*4; tmpB = tmpA + C;
    mm(RAP regref=tmpB)`.  The host ships P's value already as the
    absolute byte address (r0*4 + CBASE), so both ALUs are deleted, the
    RAP is re-targeted at P, and any per-mm base difference (C - CBASE,
    e.g. the second PSUM half) is folded into the RAP's static element
    offset.  Returns {pin_name: CBASE}."""
    # pass 1: find each dyn matmul's (alu1, alu2, pin, C)
    matches = []
    for fn in nc.m.functions:
        for bb in fn.blocks:
            writer = {}
            for ins in bb.instructions:
                if isinstance(ins, mybir.InstRegisterAlu):
                    writer[ins.outs[0].regref] = ins
                    continue
                if not (isinstance(ins, mybir.InstMatmult) and ins.outs):
                    continue
                o = ins.outs[0]
                if not (isinstance(o, mybir.RegisterAccessPattern)
                        and "_tmp_" in o.regref):
                    continue
                alu2 = writer[o.regref]
                in_reg = [a for a in alu2.ins
                          if isinstance(a, mybir.RegisterAccess)]
                in_imm = [a for a in alu2.ins
                          if isinstance(a, mybir.ImmediateValue)]
                alu1 = writer[in_reg[0].regref]
                p_reg = [a for a in alu1.ins
                         if isinstance(a, mybir.RegisterAccess)][0].regref
                assert p_reg in pin_names, f"unexpected base reg {p_reg}"
                matches.append((bb, o, alu1, alu2, p_reg,
                                int(in_imm[0].value)))
    consts = {}
    for _, _, _, _, p_reg, c_val in matches:
        consts[p_reg] = min(consts.get(p_reg, c_val), c_val)
    dead_by_bb = {}
    for bb, o, alu1, alu2, p_reg, c_val in matches:
        delta = c_val - consts[p_reg]
        assert delta == 0, f"psum base mismatch for {p_reg}: {delta}"
        o.regref = p_reg
        dead_by_bb.setdefault(id(bb), (bb, set()))[1].update(
            (alu1.name, alu2.name))
    for bb, dead in dead_by_bb.values():
        bb.instructions = [i for i in bb.instructions if i.name not in dead]
    return consts


FOLD_LDW = os.environ.get("BASS4_FOLD_LDW", "0") == "1"


def _fold_ldweights(nc):
    """Delete standalone InstLdweights for the tiny dynamic matmuls and mark
    the paired matmul self-loading, halving PE queue slots.  Waits on the
    deleted Ldweights are merged into the matmul's sync_info."""
    n = 0
    for fn in nc.m.functions:
        for bb in fn.blocks:
            pending = {}  # weights AP key -> ldweights inst
            dead = set()
            for ins in bb.instructions:
                if isinstance(ins, mybir.InstLdweights):
                    key = str(ins.ins[0])
                    pending[key] = ins
                    continue
                if isinstance(ins, mybir.InstMatmult) and len(ins.ins) == 2:
                    key = str(ins.ins[1])
                    ldw = pending.pop(key, None)
                    if ldw is None:
                        continue
                    lsi = ldw.sync_info
                    msi = ins.sync_info
                    waits = list(lsi.on_wait) if lsi else []
                    if msi is not None:
                        waits += list(msi.on_wait)
                    ups = list(msi.on_update) if msi else []
                    if lsi is not None:
                        ups += list(lsi.on_update)
                    ins.sync_info = mybir.SyncInfo(on_wait=waits,
                                                  on_update=ups)
                    ins.ldweights = True
                    dead.add(ldw.name)
                    n += 1
            if dead:
                bb.instructions = [
                    i for i in bb.instructions if i.name not in dead
                ]
    return n


def _split_excess_waits(nc):
    n = 0
    for fn in nc.m.functions:
        for bb in fn.blocks:
            il = bb.instructions
            out = []
            changed = False
            for ins in il:
                si = ins.sync_info
                if si is not None and len(si.on_wait) > 1:
                    waits = list(si.on_wait)
                    for w in waits[:-1]:
                        n += 1
                        nop = mybir.InstNoOp(
                            name=f"I-waitsplit-{n}", ins=[], outs=[]
                        )
                        nop.engine = ins.engine
                        nop.sync_info = mybir.SyncInfo(
                            on_wait=[w], on_update=[]
                        )
                        nc.register_instruction(nop)
                        out.append(nop)
                    ins.sync_info = mybir.SyncInfo(
                        on_wait=[waits[-1]], on_update=list(si.on_update)
                    )
                    changed = True
                out.append(ins)
            if changed:
                bb.instructions = out


# ---------------------------------------------------------------------------
# Host-side math


def _rot6d(alignment):
    a1, a2 = alignment[:, :3], alignment[:, 3:]
    b1 = a1 / (np.linalg.norm(a1, axis=-1, keepdims=True) + 1e-8)
    a2p = a2 - np.sum(b1 * a2, axis=-1, keepdims=True) * b1
    b2 = a2p / (np.linalg.norm(a2p, axis=-1, keepdims=True) + 1e-8)
    b3 = np.cross(b1, b2)
    return np.stack([b1, b2, b3], axis=1)


def _conv_matrix(g1, n):
    m = np.zeros((n, n), np.float64)
    for i in range(n):
        for u in range(KSIZE):
            j = i + u - KSIZE // 2
            if 0 <= j < n:
                m[i, j] += g1[u]
    return m


DFT_NAMES = [
    "wgy_t_r", "wgy_t_i",
    "wgx_t_r", "wgx_t_i", "wgx_t_in",
    "wit_r", "wit_i", "wit_in",
]


def _dft_consts(gauss_kernel):
    u, s, vt = np.linalg.svd(gauss_kernel.astype(np.float64))
    gy = np.sqrt(s[0]) * u[:, 0]
    gx = np.sqrt(s[0]) * vt[0, :]
    if gy[KSIZE // 2] < 0:
        gy, gx = -gy, -gx
    k = np.arange(XS)
    w = np.exp(-2j * np.pi * np.outer(k, k) / XS)
    winv = np.conj(w) / XS
    # NOTE: the kernel computes the TRANSPOSED image pipeline; since the
    # conv/DFT operators are symmetric matrices this only swaps the roles
    # of gx and gy below.
    wgy_t = (w @ _conv_matrix(gx, XS)).T
    wgx_t = (w @ _conv_matrix(gy, XS)).T
    wit = winv.T
    consts = {
        "wgy_t_r": np.real(wgy_t),
        "wgy_t_i": np.imag(wgy_t),
        "wgx_t_r": np.real(wgx_t),
        "wgx_t_i": np.imag(wgx_t),
        "wgx_t_in": -np.imag(wgx_t),
        "wit_r": np.real(wit),
        "wit_i": np.imag(wit),
        "wit_in": -np.imag(wit),
    }
    return {
        name: np.ascontiguousarray(m.reshape(2, 128, XS).astype(np.float32))
        for name, m in consts.items()
    }


# ---------------------------------------------------------------------------
# Device program

_PROGRAM = None


_R0_CONSTS = None  # {h: psum byte base}, set by build_program


def build_program(img_per_core=IMG_PER_CORE, n_chunks=CH):
    global _R0_CONSTS
    _patch_tile_drain()
    _patch_commit_free_tmps()
    nc = bass.Bass()

    pxp = nc.declare_dram_parameter("pxp", [img_per_core, 128, n_chunks], F32,
                                    isOutput=False)
    tpl = nc.declare_dram_parameter("tpl", [img_per_core, 128, n_chunks * W],
                                    BF16, isOutput=False)
    r0p = nc.declare_dram_parameter(
        "r0p", [img_per_core, 1, 2 * (n_chunks // G)], I32, isOutput=False
    )
    iota16 = nc.declare_dram_parameter("iota16", [2, 128, XS], BF16,
                                       isOutput=False)
    iota32 = nc.declare_dram_parameter("iota32", [128, XS], F32,
                                       isOutput=False)
    iota2 = nc.declare_dram_parameter("iota2", [128, 2 * XS], BF16,
                                      isOutput=False)
    ctf = nc.declare_dram_parameter(
        "ctf", [img_per_core, 2, 128, XS], BF16, isOutput=False
    )
    dft = {
        name: nc.declare_dram_parameter(name, [2, 128, XS], STAGE_DT,
                                        isOutput=False)
        for name in DFT_NAMES
    }
    out = nc.declare_dram_parameter(
        "out", [img_per_core, XS, XS], BF16, isOutput=True
    )

    pins = [nc.tensor.alloc_register(f"dynP{h}") for h in range(2)]
    pin_names = {p.name for p in pins}
    pvals = [
        bass.make_scalar_value(
            bass.RegisterHandles(p), min_val=0, max_val=XS - W
        )
        for p in pins
    ]

    with TileContext(nc) as tc:
        with (
            tc.tile_pool(name="const", bufs=1) as cpool,
            tc.tile_pool(name="planes", bufs=2) as ppool,
            tc.tile_pool(name="r0pool", bufs=max(img_per_core, 2)) as rpool,
            tc.tile_pool(name="deriv", bufs=2) as dpool,
            tc.tile_pool(name="build", bufs=BUFS) as bpool,
            tc.tile_pool(name="stage", bufs=2) as spool,
            tc.tile_pool(name="psumS", bufs=1, space="PSUM") as qspool,
            tc.tile_pool(name="psumA", bufs=3, space="PSUM") as qapool,
            tc.tile_pool(name="psumB", bufs=3, space="PSUM") as qbpool,
        ):
            io16 = cpool.tile([128, XS], BF16, tag="io16", name="io16")
            nc.sync.dma_start(out=io16[:], in_=iota16[0])
            io16m1 = cpool.tile([128, XS], BF16, tag="io16m1", name="io16m1")
            nc.sync.dma_start(out=io16m1[:], in_=iota16[1])
            io32 = cpool.tile([128, XS], F32, tag="io32", name="io32")
            nc.sync.dma_start(out=io32[:], in_=iota32[:])
            io2 = cpool.tile([128, 2 * XS], BF16, tag="io2", name="io2")
            nc.sync.dma_start(out=io2[:], in_=iota2[:])
            dft_t = {}
            for name in DFT_NAMES:
                for kc in range(2):
                    t = cpool.tile([128, XS], STAGE_DT, tag=f"{name}{kc}",
                                   name=f"c_{name}{kc}")
                    nc.sync.dma_start(out=t[:], in_=dft[name][kc])
                    dft_t[name, kc] = t

            for b in range(img_per_core):
                px_t = ppool.tile([128, n_chunks], F32, tag="px", name="px_t")
                nc.sync.dma_start(out=px_t[:], in_=pxp[b])
                t_t = ppool.tile([128, n_chunks * W], BF16, tag="tpl",
                                 name="t_t")
                nc.sync.dma_start(out=t_t[:], in_=tpl[b])
                r0_t = rpool.tile([1, 2 * (n_chunks // G)], I32, tag="r0",
                                  name="r0_t")
                nc.sync.dma_start(out=r0_t[:], in_=r0p[b])
                ctf_t = [ppool.tile([128, XS], BF16, tag=f"ctf{h}",
                                    name=f"ctf_t{h}") for h in range(2)]
                for h in range(2):
                    nc.sync.dma_start(out=ctf_t[h][:], in_=ctf[b, h])

                # ---- splat into transposed image (cols as partitions) ----
                img_ps = [
                    qspool.tile([128, XS], F32, tag=f"psS{h}",
                                name=f"img_ps{h}")
                    for h in range(2)
                ]
                for h in range(2):
                    nc.vector.memset(img_ps[h][:], 0.0)

                # ---- derive pxn = -px (ACT Abs bias) and, for the one-hot
                # chunks, x0 = floor(px) plus NEGATED fractional weights
                # (the t plane ships negated)
                pxn = dpool.tile([128, n_chunks], F32, tag="pxn", name="pxn")
                nc.vector.tensor_scalar(pxn[:], px_t[:], -1.0, None, ALU.mult)
                if OH_DEN <= n_chunks:
                    q = dpool.tile([128, n_chunks], F32, tag="q", name="q")
                    nc.vector.tensor_scalar(q[:], px_t[:], 0.499, BIG,
                                            ALU.subtract, ALU.add)
                    x0 = dpool.tile([128, n_chunks], F32, tag="x0",
                                    name="x0")
                    nc.vector.tensor_scalar(x0[:], q[:], BIG, None,
                                            ALU.subtract)
                    b1n = dpool.tile([128, n_chunks], F32, tag="b1n",
                                     name="b1n")
                    nc.vector.tensor_sub(b1n[:], x0[:], px_t[:])
                    b0n = dpool.tile([128, n_chunks], F32, tag="b0n",
                                     name="b0n")
                    nc.vector.tensor_scalar(b0n[:], b1n[:], -1.0, -1.0,
                                            ALU.mult, ALU.add)

                for c in range(n_chunks):
                    if c % OH_DEN == OH_DEN - 1:
                        # DVE one-hot pair with negated weights: the
                        # double negative vs the negated t plane cancels.
                        x1 = bpool.tile([128, XS], BF16, tag="x1", name="x1")
                        nc.vector.tensor_scalar(
                            x1[:], io16[:], x0[:, c : c + 1],
                            b0n[:, c : c + 1], ALU.is_equal, ALU.mult,
                        )
                        x2 = bpool.tile([128, XS], BF16, tag="x2", name="x2")
                        nc.vector.tensor_scalar(
                            x2[:], io16m1[:], x0[:, c : c + 1],
                            b1n[:, c : c + 1], ALU.is_equal, ALU.mult,
                        )
                        xts = [x1, x2]
                    else:
                        # hybrid hat: ACT computes a=|io-px|; one DVE op
                        # computes min(a,1)-1 = -hat; negated t plane
                        # restores the sign in the matmul.
                        xa = bpool.tile([128, XS], BF16, tag="xa", name="xa")
                        nc.scalar.activation(
                            xa[:], io32[:], AF.Abs,
                            bias=pxn[:, c : c + 1], scale=1.0,
                        )
                        xh = bpool.tile([128, XS], BF16, tag="xh", name="xh")
                        nc.vector.tensor_scalar(
                            xh[:], xa[:], 1.0, 1.0, ALU.min, ALU.subtract,
                        )
                        xts = [xh]
                    if c % G == 0:
                        g = c // G
                        nc.tensor.reg_load(
                            pins, r0_t[0:1, 2 * g : 2 * g + 2])
                    tcol = t_t[:, c * W : (c + 1) * W]
                    for xt in xts:
                        for h in range(2):
                            nc.tensor.matmul(
                                img_ps[h][:, bass.ds(pvals[h], W)],
                                xt[:, h * 128 : (h + 1) * 128],
                                tcol,
                                start=False,
                                stop=False,
                                skip_group_check=True,
                            )

                img_sb = [
                    spool.tile([128, XS], STAGE_DT, tag=f"isb{h}",
                               name=f"isb{h}") for h in range(2)
                ]
                for h in range(2):
                    nc.vector.tensor_copy(img_sb[h][:], img_ps[h][:])

                # ---- DFT chain (operates on transposed image) ----
                def product(terms, tag, ps_tag, mult_by=None):
                    res = []
                    for ho in range(2):
                        qp = qapool if ps_tag == "psA" else qbpool
                        ps = qp.tile([128, XS], F32, tag=ps_tag,
                                     name=f"ps_{tag}{ho}")
                        nmm = 2 * len(terms)
                        i = 0
                        for lhs_tiles, rhs_name in terms:
                            for kc in range(2):
                                nc.tensor.matmul(
                                    ps[:],
                                    lhs_tiles[kc][
                                        :, ho * 128 : (ho + 1) * 128
                                    ],
                                    dft_t[rhs_name, kc][:],
                                    start=(i == 0),
                                    stop=(i == nmm - 1),
                                )
                                i += 1
                        sb = spool.tile([128, XS], STAGE_DT,
                                        tag=f"sb{tag}{ho}",
                                        name=f"sb{tag}{ho}")
                        if mult_by is not None:
                            nc.vector.tensor_mul(sb[:], ps[:],
                                                 mult_by[ho][:])
                        else:
                            nc.vector.tensor_copy(sb[:], ps[:])
                        res.append(sb)
                    return res

                ar = product([(img_sb, "wgy_t_r")], "ar", "psB")
                ai = product([(img_sb, "wgy_t_i")], "ai", "psB")
                fr = product(
                    [(ar, "wgx_t_r"), (ai, "wgx_t_in")], "fr", "psA",
                    mult_by=ctf_t,
                )
                fi = product(
                    [(ar, "wgx_t_i"), (ai, "wgx_t_r")], "fi", "psA",
                    mult_by=ctf_t,
                )
                br = product([(fr, "wit_r"), (fi, "wit_in")], "br", "psB")
                bi = product([(fr, "wit_i"), (fi, "wit_r")], "bi", "psB")
                for ho in range(2):
                    ps = qapool.tile([128, XS], F32, tag="psA",
                                    name=f"ps_o{ho}")
                    i = 0
                    for lhs_tiles, rhs_name in [(br, "wit_r"), (bi, "wit_in")]:
                        for kc in range(2):
                            nc.tensor.matmul(
                                ps[:],
                                lhs_tiles[kc][:, ho * 128 : (ho + 1) * 128],
                                dft_t[rhs_name, kc][:],
                                start=(i == 0),
                                stop=(i == 3),
                            )
                            i += 1
                    osb = spool.tile([128, XS], BF16, tag=f"osb{ho}",
                                     name=f"osb{ho}")
                    nc.vector.tensor_copy(osb[:], ps[:])
                    nc.sync.dma_start(
                        out=out[b, ho * 128 : (ho + 1) * 128, :], in_=osb[:]
                    )
    if STRIP:
        consts = _strip_dyn_alus(nc, pin_names)
        _R0_CONSTS = {h: consts[pins[h].name] for h in range(2)}
    else:
        _R0_CONSTS = {0: 0, 1: 0}  # registers hold element offsets
    if FOLD_LDW:
        _fold_ldweights(nc)
    _split_excess_waits(nc)
    return nc


# ---------------------------------------------------------------------------
# Host prep


def _prep_image(px, py, values, n_chunks):
    """Sort one image's points by y0, pack into row-window chunks.

    Returns px_pl [128, CH] f32, t_pl [128, CH*W] bf16, r0_pl [1, CH] i32.
    """
    y0 = np.floor(py).astype(np.int32)
    fy = (py - y0).astype(np.float32)
    order = np.argsort(y0, kind="stable")
    y0s = y0[order]
    pxs = px[order].astype(np.float32)
    fys = fy[order]
    vs = values[order].astype(np.float32)

    n = y0s.shape[0]
    # first index whose y0 >= y0s[i] + (W - 1): a group starting at i must
    # end before it so taps stay inside [y0s[i], y0s[i] + W)
    limit = np.searchsorted(y0s, y0s + (W - 1), side="left")
    # greedy G-chunk groups sharing one r0
    gstarts = []
    gends = []
    s = 0
    while s < n:
        e = min(s + 128 * G, int(limit[s]), n)
        gstarts.append(s)
        gends.append(e)
        s = e
    ngroups = len(gstarts)
    n_groups_max = n_chunks // G
    assert ngroups <= n_groups_max, (
        f"chunking overflow: {ngroups} > {n_groups_max}")

    # split each group into G chunk slots
    starts, ends, group_of_chunk = [], [], []
    for g, (gs, ge) in enumerate(zip(gstarts, gends)):
        for k in range(G):
            cs = min(gs + 128 * k, ge)
            ce = min(cs + 128, ge)
            starts.append(cs)
            ends.append(ce)
            group_of_chunk.append(g)
    nch = len(starts)

    starts_a = np.asarray(starts, np.int64)
    ends_a = np.asarray(ends, np.int64)
    lens = ends_a - starts_a
    chunk_id = np.repeat(np.arange(nch, dtype=np.int64), lens)
    slot = np.arange(n, dtype=np.int64) - np.repeat(starts_a, lens)

    gr0 = np.minimum(y0s[np.asarray(gstarts, np.int64)],
                     XS - W).astype(np.int32)
    # interleaved pairs (h0, h1) per GROUP: absolute PSUM byte addresses
    # (STRIP mode) or plain element offsets (no-strip mode)
    r0_pl = np.zeros((1, 2 * n_groups_max), np.int32)
    for h in range(2):
        r0_pl[0, h::2] = _R0_CONSTS[h]
        r0_pl[0, h : 2 * ngroups : 2] += gr0 * (4 if STRIP else 1)

    px_pl = np.zeros((128, n_chunks), np.float32)
    px_pl[slot, chunk_id] = pxs

    r0 = gr0[np.asarray(group_of_chunk, np.int64)]
    j = (y0s - r0[chunk_id]).astype(np.int64)
    w0 = (1.0 - fys) * vs
    w1 = fys * vs
    t3 = np.zeros((128, n_chunks, W), np.float32)
    t3[slot, chunk_id, j] = w0
    m = j + 1 <= W - 1
    t3[slot[m], chunk_id[m], j[m] + 1] = w1[m]
    # negated: the device computes -hat on the x side (see build_program)
    t_pl = np.ascontiguousarray(-t3.reshape(128, n_chunks * W)).astype(NPBF16)
    return px_pl, t_pl, r0_pl


def _prep_host(alignment, shifts, coords, values, gauss_kernel, ctf,
               img_per_core=IMG_PER_CORE, n_chunks=CH):
    rot = _rot6d(alignment.astype(np.float64))
    nb = rot.shape[0]
    half = XS // 2
    pc = coords.astype(np.float64)
    vals = values.astype(np.float32)

    pxs_all = []
    tpl_all = []
    r0_all = []
    for b in range(nb):
        px = pc @ rot[b, 0] + (float(shifts[b, 0]) + half)
        py = pc @ rot[b, 1] + (float(shifts[b, 1]) + half)
        np.clip(px, 0.0, XS - 1.0, out=px)
        np.clip(py, 0.0, XS - 1.0, out=py)
        px_pl, t_pl, r0_pl = _prep_image(
            px.astype(np.float32), py.astype(np.float32), vals, n_chunks
        )
        pxs_all.append(px_pl)
        tpl_all.append(t_pl)
        r0_all.append(r0_pl)

    iota = np.arange(XS, dtype=np.float64)
    iota16 = np.ascontiguousarray(
        np.stack([
            np.broadcast_to(iota, (128, XS)),
            np.broadcast_to(iota - 1.0, (128, XS)),
        ]).astype(NPBF16)
    )
    iota32 = np.ascontiguousarray(
        np.broadcast_to(iota, (128, XS)).astype(np.float32)
    )
    iota2 = np.ascontiguousarray(
        np.broadcast_to(np.concatenate([iota, iota]), (128, 2 * XS))
        .astype(NPBF16)
    )
    consts = _dft_consts(gauss_kernel)
    # transposed-image pipeline -> ship the CTF transposed
    cs = np.fft.ifftshift(ctf.astype(np.float32), axes=(-2, -1))
    cs = np.ascontiguousarray(np.transpose(cs, (0, 2, 1)))
    cs = np.ascontiguousarray(cs.reshape(nb, 2, 128, XS)).astype(NPBF16)

    n_cores = nb // img_per_core
    in_maps = []
    for core in range(n_cores):
        sl = slice(core * img_per_core, (core + 1) * img_per_core)
        m = {
            "pxp": np.ascontiguousarray(np.stack(pxs_all[sl])),
            "tpl": np.ascontiguousarray(np.stack(tpl_all[sl])),
            "r0p": np.ascontiguousarray(np.stack(r0_all[sl])),
            "iota16": iota16, "iota32": iota32, "iota2": iota2,
            "ctf": np.ascontiguousarray(cs[sl]),
        }
        m.update(consts)
        in_maps.append(m)
    return in_maps


def kernel(alignment, shifts, coords, values, gauss_kernel, ctf):
    global _PROGRAM
    if _PROGRAM is None:
        _PROGRAM = build_program()
    in_maps = _prep_host(
        np.asarray(alignment), np.asarray(shifts), np.asarray(coords),
        np.asarray(values), np.asarray(gauss_kernel), np.asarray(ctf),
    )
    res = run_bass_kernel_spmd(_PROGRAM, in_maps, list(range(N_CORES)))
    out_t = np.concatenate([r["out"] for r in res.results], axis=0)
    return np.ascontiguousarray(
        np.transpose(out_t, (0, 2, 1))
    ).astype(np.float32)
